# revision 13
# baseline (speedup 1.0000x reference)
"""Paged GQA decode attention (sparse_attention) on 8 TRN2 NeuronCores.

Sharding: tensor-parallel by KV head (8 heads -> 8 cores). Each core gets its
head's slice of the KV pool, pre-merged on host into single bf16 rows
[khi(128) | vhi(128)] (512 B) so that ONE natural dma_gather per (group,
pool-half) fetches both K and V for a token: half the HBM bytes and half the
descriptors of a hi/lo split scheme, at the 512 B descriptor size that
avoids the sub-512B DMA latency penalty.

Requests are packed CONTIGUOUSLY inside each (group, pool-half) gather (no
per-request 128-padding): a 128-token slot shared by several requests gets
one score "subslot" per request, and the foreign partitions of a shared
slot are zeroed via a per-partition bias on the ACT exp (exp(s-50) ~ 0).
Only each gather block's final slot carries pad tokens, which read a zeroed
spare pool row (K=0 -> exp(0)=1, V=0) and are subtracted from the softmax
sum on host; if a pool half has no spare row to zero (only possible when
every row is referenced), those tails fall back to bias columns as well.

Per core dataflow (fully specialized at build time on the actual seq_lens /
pool-half split, identical across cores):
  gather: kv[tok, 0:128]=K, kv[tok, 128:256]=V  (natural layout, tok on
          partitions, one 512 B descriptor per token)
  K^T:    per 128-token slot, PE transpose K chunk -> PSUM (bf16), batched
          8 slots/bank; PSUM->SBUF copies split DVE/ACT
  QK:     scores^T[tok,4] = ktT @ (qhi|qlo) per subslot (4-col streams)
  exp:    one ACT Exp per group -> p^T in SBUF directly as bf16, then tiny
          re-exps with bias columns for shared/boundary subslots
  PV:     o^T[d,4] accum with V-natural stationary, p as 4-col moving
  sums:   ones-vector matmul -> per-subslot partial sums; final reduction
          and softmax normalization happen on host.
"""

import os

import numpy as np
import ml_dtypes

import concourse.bacc as bacc
import concourse.bass as bass
import concourse.mybir as mybir
import concourse.tile as tile
from concourse.bass_utils import run_bass_kernel_spmd

B, S, HQ, HKV, D = 32, 2048, 32, 8, 128
G = HQ // HKV
POOL = B * S
HALF = POOL // 2
SCALE = D ** -0.5
NCORES = 8
# variable group sizes: shrinking tail groups keep the drain short; big
# first and middle groups keep the gather pipeline busy
GSIZES = (4, 4, 4, 4, 4, 4, 3, 2, 2, 1)
GROUPS = len(GSIZES)
GOFF = tuple(int(np.sum(GSIZES[:g])) for g in range(GROUPS + 1))
TB = 8             # K^T transpose slots per PSUM bank / copy batch
NEG = -50.0        # bias for foreign partitions: exp(s-50) ~ 0

BF16 = ml_dtypes.bfloat16

_prog_cache: dict = {}
LAST_RESULT = None  # test.py introspection (exec time etc.)


def _pad128(n):
    return (n + 127) // 128 * 128


def _layout(meta, mask_halves):
    """meta[g][h][j] = valid token count of request j in half h of group g.

    Sections are packed contiguously per (group, half); returns per group:
      nslots        gather slots (lo half first, then hi)
      subs          [(slot, owner j, bias_col_id or -1)] per score subslot
      req_subs[j]   ordered global subslot ids owned by j
      req_ranges[j] contiguous (sub0, cnt) ranges in subslot units
      req_pads[j]   pad tokens to subtract from j's softmax sums on host
    plus bias column specs [(part_lo, part_hi)] and idx/output offsets.
    """
    info = []
    bias_cols = []  # (part_lo, part_hi): keep [lo,hi), NEG elsewhere
    icol = 0
    for g in range(GROUPS):
        sz = GSIZES[g]
        subs = []          # (gslot, j, bias_id)
        req_subs = [[] for _ in range(sz)]
        req_pads = [0] * sz
        halves = []
        slot_base = 0
        for h, secs in enumerate(meta[g]):
            n = int(np.sum(secs))
            P = _pad128(n)
            halves.append(dict(n=P, real=n, secs=secs, ioff=icol,
                               slots=P // 128))
            icol += P // 16
        for h in (0, 1):
            hh = halves[h]
            secs = hh["secs"]
            n = hh["real"]
            c0 = np.concatenate([[0], np.cumsum(secs)]).astype(int)
            for s in range(hh["slots"]):
                lo, hi = 128 * s, 128 * s + 128
                owners = [j for j in range(sz)
                          if c0[j] < hi and c0[j + 1] > lo and secs[j] > 0]
                has_pad = hi > n
                for j in owners:
                    plo, phi = max(c0[j], lo) - lo, min(c0[j + 1], hi) - lo
                    whole = plo == 0 and (phi == 128 or
                                          (has_pad and phi == n - lo))
                    if len(owners) == 1 and whole and not (
                            has_pad and mask_halves[h]):
                        bid = -1
                        if has_pad:  # zero-row pads: exp(0)=1, host subtracts
                            req_pads[j] += hi - n
                    else:
                        bid = len(bias_cols)
                        bias_cols.append((plo, phi))
                    req_subs[j].append(len(subs))
                    subs.append((slot_base + s, j, bid))
            slot_base += hh["slots"]
        req_ranges = []
        for j in range(sz):
            ranges = []
            for si in req_subs[j]:
                if ranges and si == ranges[-1][0] + ranges[-1][1]:
                    ranges[-1][1] += 1
                else:
                    ranges.append([si, 1])
            req_ranges.append([tuple(r) for r in ranges])
        info.append(dict(halves=halves, nslots=slot_base, subs=subs,
                         req_subs=req_subs, req_ranges=req_ranges,
                         req_pads=req_pads, nsub=len(subs)))
    # output packing: o^T cols per group at 4*GOFF[g]; all sums cols in one
    # partition-0 row segment starting at col 4*B (single final DMA)
    sb = 0
    for g, gi in enumerate(info):
        gi["obase"] = 4 * GOFF[g]
        gi["sbase"] = 4 * B + sb
        sb += 4 * gi["nsub"]
    return info, bias_cols, icol, 4 * B + sb


def _build_program(meta, mask_halves):
    info, bias_cols, idx_w, out_w = _layout(meta, mask_halves)
    n_bias = max(1, len(bias_cols))
    dt = mybir.dt
    nc = bacc.Bacc(trn_type="TRN2")

    kv_il = nc.dram_tensor("kv_il", [POOL, 256], dt.bfloat16, kind="ExternalInput")
    qhiT = nc.dram_tensor("qhiT", [128, 128], dt.bfloat16, kind="ExternalInput")
    qloT = nc.dram_tensor("qloT", [128, 128], dt.bfloat16, kind="ExternalInput")
    identd = nc.dram_tensor("identd", [128, 128], dt.bfloat16, kind="ExternalInput")
    biasd = nc.dram_tensor("biasc", [1, 128 * n_bias], dt.bfloat16,
                           kind="ExternalInput")
    idx_w = max(1, idx_w)
    idx_d = nc.dram_tensor("idx_all", [128, idx_w], dt.int16, kind="ExternalInput")
    o_dram = nc.dram_tensor("o_un", [128, max(1, out_w)], dt.float32,
                            kind="ExternalOutput")

    with tile.TileContext(nc) as tc:
        with (
            tc.tile_pool(name="const", bufs=1) as cpool,
            tc.tile_pool(name="kv", bufs=4) as kvp,
            tc.tile_pool(name="ktT", bufs=2) as ktp,
            tc.tile_pool(name="pt", bufs=2) as ptp,
            tc.tile_pool(name="stg", bufs=2) as stgp,
            tc.tile_pool(name="ps_kt", bufs=3, space="PSUM") as pskt,
            tc.tile_pool(name="ps_sc", bufs=2, space="PSUM") as pssc,
            tc.tile_pool(name="ps_pv", bufs=2, space="PSUM") as pspv,
        ):
            qhi_t = cpool.tile([128, 128], dt.bfloat16, tag="qhi")
            qlo_t = cpool.tile([128, 128], dt.bfloat16, tag="qlo")
            ident_t = cpool.tile([128, 128], dt.bfloat16, tag="ident")
            ones_t = cpool.tile([128, 1], dt.bfloat16, tag="ones")
            bias_t = cpool.tile([1, 128 * n_bias], dt.bfloat16, tag="biasc")
            ones4_t = cpool.tile([1, 4], dt.bfloat16, tag="ones4")
            sums_t = cpool.tile([1, max(4, out_w - 4 * B)], dt.float32,
                                tag="sumsrow")
            idx_t = cpool.tile([128, idx_w], dt.int16, tag="idxall")
            # group-0 idx first (unblocks gather 0), then the small constant
            # uploads (ident gates the first PE transpose!), then the rest
            def _idx_dma(g):
                for h in (0, 1):
                    hh = info[g]["halves"][h]
                    n = hh["n"]
                    if n == 0:
                        continue
                    i0 = hh["ioff"]
                    nc.sync.dma_start(out=idx_t[:, i0:i0 + n // 16],
                                      in_=idx_d[:, i0:i0 + n // 16])
            _idx_dma(0)
            nc.sync.dma_start(out=ident_t[:], in_=identd[:])
            nc.sync.dma_start(out=qhi_t[:], in_=qhiT[:])
            nc.sync.dma_start(out=qlo_t[:], in_=qloT[:])
            nc.sync.dma_start(out=bias_t[:], in_=biasd[:])
            for g in range(1, GROUPS):
                _idx_dma(g)
            nc.vector.memset(ones_t[:], 1.0)
            nc.vector.memset(ones4_t[:], 1.0)

            for g in range(GROUPS):
                gi = info[g]
                nslots, nsub = gi["nslots"], gi["nsub"]
                ncols = 4 * nsub
                OC = G * GSIZES[g]
                ob = gi["obase"]
                if nslots == 0:
                    # all requests in this group are empty (degenerate input)
                    z = stgp.tile([128, OC], dt.float32, tag="ostg")
                    nc.vector.memset(z[:], 0.0)
                    nc.sync.dma_start(out=o_dram[:, ob:ob + OC], in_=z[:])
                    continue
                # --- one merged K|V gather per pool half ------------------
                kvt = kvp.tile([128, nslots, 256], dt.bfloat16, tag="kv")
                n_lo_slots = gi["halves"][0]["slots"]
                for h in (0, 1):
                    n = gi["halves"][h]["n"]
                    if n == 0:
                        continue
                    ioff = gi["halves"][h]["ioff"]
                    it = idx_t[:, ioff:ioff + n // 16]
                    src = kv_il[0:HALF, :] if h == 0 else kv_il[HALF:POOL, :]
                    sb = 0 if h == 0 else n_lo_slots
                    nc.gpsimd.dma_gather(
                        out_ap=kvt[:, sb:sb + n // 128, :], in_ap=src,
                        idxs_ap=it, num_idxs=n, num_idxs_reg=n, elem_size=256,
                        transpose=False, single_packet=False)

                # --- K^T: PE transpose batches + PSUM->SBUF copies ---------
                ktT = ktp.tile([128, nslots * 128], dt.bfloat16, tag="ktT")
                for bi, s0 in enumerate(range(0, nslots, TB)):
                    nb = min(TB, nslots - s0)
                    kt_ps = pskt.tile([128, TB * 128], dt.bfloat16, tag="ktps")
                    for i in range(nb):
                        nc.tensor.transpose(kt_ps[:, 128 * i:128 * (i + 1)],
                                            kvt[:, s0 + i, 0:128], ident_t[:])
                    dst = ktT[:, 128 * s0:128 * (s0 + nb)]
                    if bi % 4 == 3:
                        nc.scalar.activation(dst, kt_ps[:, 0:128 * nb],
                                             mybir.ActivationFunctionType.Copy)
                    else:
                        nc.vector.tensor_copy(out=dst, in_=kt_ps[:, 0:128 * nb])

                # --- QK: scores^T per subslot into one PSUM bank ----------
                sc = pssc.tile([128, ncols], dt.float32, tag="sc")
                for si, (s, j, bid) in enumerate(gi["subs"]):
                    b = GOFF[g] + j
                    kT = ktT[:, 128 * s:128 * (s + 1)]
                    out = sc[:, 4 * si:4 * si + 4]
                    if bid >= 0:  # seed foreign partitions with -50 (K=1 mm)
                        nc.tensor.matmul(out,
                                         bias_t[0:1, 128 * bid:128 * bid + 128],
                                         ones4_t[0:1, :],
                                         start=True, stop=False)
                    nc.tensor.matmul(out, kT, qhi_t[:, 4 * b:4 * b + 4],
                                     start=(bid < 0), stop=False)
                    nc.tensor.matmul(out, kT, qlo_t[:, 4 * b:4 * b + 4],
                                     start=False, stop=True)

                # --- softmax numerator, straight to bf16 (scores are O(1))
                pt = ptp.tile([128, ncols], dt.bfloat16, tag="pt")
                pvs = pspv.tile([128, OC + ncols], dt.float32, tag="pvs")
                nc.scalar.activation(pt[:], sc[:],
                                     mybir.ActivationFunctionType.Exp)

                # --- PV (o^T accum, V-natural stationary) + sums ----------
                for j in range(GSIZES[g]):
                    rsubs = gi["req_subs"][j]
                    oc = G * j
                    if not rsubs:
                        nc.vector.memset(pvs[:, oc:oc + G], 0.0)
                        continue
                    last = len(rsubs) - 1
                    for kk, si in enumerate(rsubs):
                        s = gi["subs"][si][0]
                        nc.tensor.matmul(pvs[:, oc:oc + G], kvt[:, s, 128:256],
                                         pt[:, 4 * si:4 * si + 4],
                                         start=(kk == 0), stop=(kk == last))
                    for (s0, cnt) in gi["req_ranges"][j]:
                        nc.tensor.matmul(
                            pvs[0:1, OC + 4 * s0:OC + 4 * (s0 + cnt)],
                            ones_t[:, 0:1], pt[:, 4 * s0:4 * (s0 + cnt)],
                            start=True, stop=True)

                ostg = stgp.tile([128, OC], dt.float32, tag="ostg")
                nc.scalar.activation(ostg[:], pvs[:, 0:OC],
                                     mybir.ActivationFunctionType.Copy)
                so = gi["sbase"] - 4 * B
                nc.scalar.activation(sums_t[0:1, so:so + ncols],
                                     pvs[0:1, OC:OC + ncols],
                                     mybir.ActivationFunctionType.Copy)
                nc.sync.dma_start(out=o_dram[:, ob:ob + OC], in_=ostg[:])

            # all per-group sums in one trailing DMA (partition-0 row)
            nsum = out_w - 4 * B
            if nsum > 0:
                nc.sync.dma_start(out=o_dram[0:1, 4 * B:out_w],
                                  in_=sums_t[0:1, 0:nsum])

    nc.compile()
    return nc, info, bias_cols


def prepare(inputs):
    q = np.asarray(inputs["q"], np.float32)
    k = np.asarray(inputs["k"], np.float32)
    v = np.asarray(inputs["v"], np.float32)
    k_buffer = np.asarray(inputs["k_buffer"], np.float32)
    v_buffer = np.asarray(inputs["v_buffer"], np.float32)
    req_to_token = np.asarray(inputs["req_to_token"])
    req_pool_indices = np.asarray(inputs["req_pool_indices"])
    seq_lens = np.asarray(inputs["seq_lens"]).astype(np.int64)
    out_cache_loc = np.asarray(inputs["out_cache_loc"]).astype(np.int64)

    # store_kv_cache scatter (tiny: 32 rows) + per-request token lists
    kb = k_buffer.copy()
    vb = v_buffer.copy()
    kb[out_cache_loc] = k.reshape(B, HKV, D)
    vb[out_cache_loc] = v.reshape(B, HKV, D)
    tok = req_to_token[req_pool_indices]

    # spare (unreferenced) pool row per half -> zeroed pad target
    referenced = np.zeros(POOL, bool)
    for b in range(B):
        referenced[tok[b, :seq_lens[b]]] = True
    free_lo = np.flatnonzero(~referenced[:HALF])
    free_hi = np.flatnonzero(~referenced[HALF:])
    zero_row = [int(free_lo[0]) if len(free_lo) else -1,
                int(free_hi[0]) + HALF if len(free_hi) else -1]
    mask_halves = (zero_row[0] < 0, zero_row[1] < 0)

    # group 0: the 4 smallest requests (fast fill); the 5 next-smallest fill
    # the shrinking tail groups (2,2,1 -> short drain); the rest biggest-first
    asc = list(np.argsort(seq_lens, kind="stable"))
    mid = asc[9:][::-1]
    tail = asc[5:7] + asc[7:9] + [asc[4]]
    order = np.array(asc[:4] + mid + tail, dtype=np.int64)

    meta = []
    idx_blocks = []
    for g in range(GROUPS):
        lo_secs, hi_secs = [], []
        for h in (0, 1):
            parts = []
            secs = lo_secs if h == 0 else hi_secs
            pad_idx = zero_row[h] - (0 if h == 0 else HALF)
            if pad_idx < 0:
                pad_idx = 0  # bias fallback half: any valid row
            for j in range(GSIZES[g]):
                b = int(order[GOFF[g] + j])
                t = tok[b, :seq_lens[b]].astype(np.int64)
                tl = t[t < HALF] if h == 0 else t[t >= HALF] - HALF
                secs.append(len(tl))
                parts.append(tl)
            full = np.concatenate(parts) if parts else np.zeros(0, np.int64)
            P = _pad128(len(full))
            if P:
                arr = np.full(P, pad_idx, np.int64)
                arr[:len(full)] = full
                # [16, n/16] wrap, replicated into all 8 GPSIMD-core stripes
                idx_blocks.append(
                    np.tile(arr.astype(np.int16).reshape(-1, 16).T, (8, 1)))
        meta.append((tuple(lo_secs), tuple(hi_secs)))
    meta = tuple(meta)
    if idx_blocks:
        idx_all = np.ascontiguousarray(np.concatenate(idx_blocks, axis=1))
    else:
        idx_all = np.zeros((128, 1), np.int16)

    key = (meta, mask_halves)
    if key not in _prog_cache:
        _prog_cache[key] = _build_program(meta, mask_halves)
    nc, info, bias_cols = _prog_cache[key]

    biasc = np.zeros((1, 128 * max(1, len(bias_cols))), BF16)
    for bi, (plo, phi) in enumerate(bias_cols):
        col = np.full(128, NEG, BF16)
        col[plo:phi] = 0.0
        biasc[0, 128 * bi:128 * bi + 128] = col

    ident = np.eye(128, dtype=BF16)
    in_maps = []
    for c in range(NCORES):
        k_hi = kb[:, c, :].astype(BF16)
        v_hi = vb[:, c, :].astype(BF16)
        qc = (q.reshape(B, HKV, G, D)[order, c] * SCALE).reshape(B * G, D)
        qT = np.ascontiguousarray(qc.T)
        q_hi = qT.astype(BF16)
        q_lo = (qT - q_hi.astype(np.float32)).astype(BF16)
        kv_core = np.concatenate([k_hi, v_hi], axis=1)
        for zr in zero_row:
            if zr >= 0:
                kv_core[zr] = 0
        im = {
            "kv_il": np.ascontiguousarray(kv_core),
            "qhiT": np.ascontiguousarray(q_hi),
            "qloT": np.ascontiguousarray(q_lo),
            "identd": ident,
            "biasc": biasc,
            "idx_all": idx_all,
        }
        in_maps.append(im)
    return nc, info, in_maps, order, mask_halves


def postprocess(results, info, order, mask_halves, cores=None):
    out = np.zeros((B, HQ, D), np.float32)
    for c in (cores if cores is not None else range(NCORES)):
        o_un = results[c]["o_un"]  # [128, W]: o^T cols + sums row segment
        for g in range(GROUPS):
            gi = info[g]
            ob = gi["obase"]
            sb = gi["sbase"]  # sums cols (partition-0 row)
            for j in range(GSIZES[g]):
                b = int(order[GOFF[g] + j])
                stot = np.zeros(G, np.float64)
                for (s0, cnt) in gi["req_ranges"][j]:
                    seg = o_un[0, sb + 4 * s0:sb + 4 * (s0 + cnt)]
                    stot += seg.astype(np.float64).reshape(cnt, G).sum(axis=0)
                stot -= gi["req_pads"][j]  # zero-row pads: exp(0)=1 each
                ov = o_un[:, ob + G * j:ob + G * (j + 1)]  # [128 d, G]
                with np.errstate(divide="ignore", invalid="ignore"):
                    out[b, c * G:(c + 1) * G, :] = (ov / stot[None, :]).T
    return out.reshape(B, HQ * D).astype(np.float32)


def kernel(**inputs):
    global LAST_RESULT
    nc, info, in_maps, order, mask_halves = prepare(inputs)
    res = run_bass_kernel_spmd(nc, in_maps, core_ids=list(range(NCORES)),
                               trace=False)
    LAST_RESULT = res
    return postprocess(res.results, info, order, mask_halves)


# revision 16
# speedup vs baseline: 1.0724x; 1.0724x over previous
"""Paged GQA decode attention (sparse_attention) on 8 TRN2 NeuronCores.

Sharding: tensor-parallel by KV head (8 heads -> 8 cores). Each core gets its
head's slice of the KV pool, pre-merged on host into single bf16 rows
[khi(128) | vhi(128)] (512 B) so that ONE natural dma_gather per (group,
pool-half) fetches both K and V for a token: half the HBM bytes and half the
descriptors of a hi/lo split scheme, at the 512 B descriptor size that
avoids the sub-512B DMA latency penalty.

Requests are packed CONTIGUOUSLY inside each (group, pool-half) gather (no
per-request 128-padding): a 128-token slot shared by several requests gets
one score "subslot" per request, and the foreign partitions of a shared
slot are zeroed via a per-partition bias on the ACT exp (exp(s-50) ~ 0).
Only each gather block's final slot carries pad tokens, which read a zeroed
spare pool row (K=0 -> exp(0)=1, V=0) and are subtracted from the softmax
sum on host; if a pool half has no spare row to zero (only possible when
every row is referenced), those tails fall back to bias columns as well.

Per core dataflow (fully specialized at build time on the actual seq_lens /
pool-half split, identical across cores):
  gather: kv[tok, 0:128]=K, kv[tok, 128:256]=V  (natural layout, tok on
          partitions, one 512 B descriptor per token)
  K^T:    per 128-token slot, PE transpose K chunk -> PSUM (bf16), batched
          8 slots/bank; PSUM->SBUF copies split DVE/ACT
  QK:     scores^T[tok,4] = ktT @ (qhi|qlo) per subslot (4-col streams)
  exp:    one ACT Exp per group -> p^T in SBUF directly as bf16, then tiny
          re-exps with bias columns for shared/boundary subslots
  PV:     o^T[d,4] accum with V-natural stationary, p as 4-col moving
  sums:   ones-vector matmul -> per-subslot partial sums; final reduction
          and softmax normalization happen on host.
"""

import os

import numpy as np
import ml_dtypes

import concourse.bacc as bacc
import concourse.bass as bass
import concourse.mybir as mybir
import concourse.tile as tile
from concourse.bass_utils import run_bass_kernel_spmd

B, S, HQ, HKV, D = 32, 2048, 32, 8, 128
G = HQ // HKV
POOL = B * S
HALF = POOL // 2
SCALE = D ** -0.5
NCORES = 8
# variable group sizes: shrinking tail groups keep the drain short; big
# first and middle groups keep the gather pipeline busy
GSIZES = (4, 5, 5, 5, 5, 5, 2, 1)
GROUPS = len(GSIZES)
GOFF = tuple(int(np.sum(GSIZES[:g])) for g in range(GROUPS + 1))
TB = 8             # K^T transpose slots per PSUM bank / copy batch
NEG = -50.0        # bias for foreign partitions: exp(s-50) ~ 0
KVBUFS = 3         # kv tile ring depth (gather lookahead)
CPMOD, CPACT = 5, 4  # K^T copy batch bi % CPMOD == CPACT -> ACT else DVE

BF16 = ml_dtypes.bfloat16

_prog_cache: dict = {}
LAST_RESULT = None  # test.py introspection (exec time etc.)


def _pad128(n):
    return (n + 127) // 128 * 128


def _layout(meta, mask_halves):
    """meta[g][h][j] = valid token count of request j in half h of group g.

    Sections are packed contiguously per (group, half); returns per group:
      nslots        gather slots (lo half first, then hi)
      subs          [(slot, owner j, bias_col_id or -1)] per score subslot
      req_subs[j]   ordered global subslot ids owned by j
      req_ranges[j] contiguous (sub0, cnt) ranges in subslot units
      req_pads[j]   pad tokens to subtract from j's softmax sums on host
    plus bias column specs [(part_lo, part_hi)] and idx/output offsets.
    """
    info = []
    bias_cols = []  # (part_lo, part_hi): keep [lo,hi), NEG elsewhere
    icol = 0
    for g in range(GROUPS):
        sz = GSIZES[g]
        subs = []          # (gslot, j, bias_id)
        req_subs = [[] for _ in range(sz)]
        req_pads = [0] * sz
        halves = []
        slot_base = 0
        for h, secs in enumerate(meta[g]):
            n = int(np.sum(secs))
            P = _pad128(n)
            halves.append(dict(n=P, real=n, secs=secs, ioff=icol,
                               slots=P // 128))
            icol += P // 16
        for h in (0, 1):
            hh = halves[h]
            secs = hh["secs"]
            n = hh["real"]
            c0 = np.concatenate([[0], np.cumsum(secs)]).astype(int)
            for s in range(hh["slots"]):
                lo, hi = 128 * s, 128 * s + 128
                owners = [j for j in range(sz)
                          if c0[j] < hi and c0[j + 1] > lo and secs[j] > 0]
                has_pad = hi > n
                for j in owners:
                    plo, phi = max(c0[j], lo) - lo, min(c0[j + 1], hi) - lo
                    whole = plo == 0 and (phi == 128 or
                                          (has_pad and phi == n - lo))
                    if len(owners) == 1 and whole and not (
                            has_pad and mask_halves[h]):
                        bid = -1
                        if has_pad:  # zero-row pads: exp(0)=1, host subtracts
                            req_pads[j] += hi - n
                    else:
                        bid = len(bias_cols)
                        bias_cols.append((plo, phi))
                    req_subs[j].append(len(subs))
                    subs.append((slot_base + s, j, bid))
            slot_base += hh["slots"]
        req_ranges = []
        for j in range(sz):
            ranges = []
            for si in req_subs[j]:
                if ranges and si == ranges[-1][0] + ranges[-1][1]:
                    ranges[-1][1] += 1
                else:
                    ranges.append([si, 1])
            req_ranges.append([tuple(r) for r in ranges])
        info.append(dict(halves=halves, nslots=slot_base, subs=subs,
                         req_subs=req_subs, req_ranges=req_ranges,
                         req_pads=req_pads, nsub=len(subs)))
    # output packing: o^T cols per group at 4*GOFF[g]; all sums cols in one
    # partition-0 row segment starting at col 4*B (single final DMA)
    sb = 0
    for g, gi in enumerate(info):
        gi["obase"] = 4 * GOFF[g]
        gi["sbase"] = 4 * B + sb
        sb += 4 * gi["nsub"]
    return info, bias_cols, icol, 4 * B + sb


def _build_program(meta, mask_halves):
    info, bias_cols, idx_w, out_w = _layout(meta, mask_halves)
    n_bias = max(1, len(bias_cols))
    dt = mybir.dt
    nc = bacc.Bacc(trn_type="TRN2")

    kv_il = nc.dram_tensor("kv_il", [POOL, 256], dt.bfloat16, kind="ExternalInput")
    qhiT = nc.dram_tensor("qhiT", [128, 128], dt.bfloat16, kind="ExternalInput")
    qloT = nc.dram_tensor("qloT", [128, 128], dt.bfloat16, kind="ExternalInput")
    identd = nc.dram_tensor("identd", [128, 128], dt.bfloat16, kind="ExternalInput")
    biasd = nc.dram_tensor("biasc", [1, 128 * n_bias], dt.bfloat16,
                           kind="ExternalInput")
    idx_w = max(1, idx_w)
    idx_d = nc.dram_tensor("idx_all", [128, idx_w], dt.int16, kind="ExternalInput")
    o_dram = nc.dram_tensor("o_un", [128, max(1, out_w)], dt.float32,
                            kind="ExternalOutput")

    with tile.TileContext(nc) as tc:
        with (
            tc.tile_pool(name="const", bufs=1) as cpool,
            tc.tile_pool(name="kv", bufs=KVBUFS) as kvp,
            tc.tile_pool(name="ktT", bufs=2) as ktp,
            tc.tile_pool(name="pt", bufs=2) as ptp,
            tc.tile_pool(name="stg", bufs=2) as stgp,
            tc.tile_pool(name="ps_kt", bufs=3, space="PSUM") as pskt,
            tc.tile_pool(name="ps_sc", bufs=2, space="PSUM") as pssc,
            tc.tile_pool(name="ps_pv", bufs=2, space="PSUM") as pspv,
        ):
            qhi_t = cpool.tile([128, 128], dt.bfloat16, tag="qhi")
            qlo_t = cpool.tile([128, 128], dt.bfloat16, tag="qlo")
            ident_t = cpool.tile([128, 128], dt.bfloat16, tag="ident")
            ones_t = cpool.tile([128, 1], dt.bfloat16, tag="ones")
            bias_t = cpool.tile([1, 128 * n_bias], dt.bfloat16, tag="biasc")
            ones4_t = cpool.tile([1, 4], dt.bfloat16, tag="ones4")
            sums_t = cpool.tile([1, max(4, out_w - 4 * B)], dt.float32,
                                tag="sumsrow")
            idx_t = cpool.tile([128, idx_w], dt.int16, tag="idxall")
            # group-0 idx first (unblocks gather 0), then the small constant
            # uploads (ident gates the first PE transpose!), then the rest
            def _idx_dma(g):
                for h in (0, 1):
                    hh = info[g]["halves"][h]
                    n = hh["n"]
                    if n == 0:
                        continue
                    i0 = hh["ioff"]
                    nc.sync.dma_start(out=idx_t[:, i0:i0 + n // 16],
                                      in_=idx_d[:, i0:i0 + n // 16])
            _idx_dma(0)
            nc.sync.dma_start(out=ident_t[:], in_=identd[:])
            nc.sync.dma_start(out=qhi_t[:], in_=qhiT[:])
            nc.sync.dma_start(out=qlo_t[:], in_=qloT[:])
            nc.sync.dma_start(out=bias_t[:], in_=biasd[:])
            for g in range(1, GROUPS):
                _idx_dma(g)
            nc.vector.memset(ones_t[:], 1.0)
            nc.vector.memset(ones4_t[:], 1.0)

            for g in range(GROUPS):
                gi = info[g]
                nslots, nsub = gi["nslots"], gi["nsub"]
                ncols = 4 * nsub
                OC = G * GSIZES[g]
                ob = gi["obase"]
                if nslots == 0:
                    # all requests in this group are empty (degenerate input)
                    z = stgp.tile([128, OC], dt.float32, tag="ostg")
                    nc.vector.memset(z[:], 0.0)
                    nc.sync.dma_start(out=o_dram[:, ob:ob + OC], in_=z[:])
                    continue
                # --- one merged K|V gather per pool half ------------------
                kvt = kvp.tile([128, nslots, 256], dt.bfloat16, tag="kv")
                n_lo_slots = gi["halves"][0]["slots"]
                for h in (0, 1):
                    n = gi["halves"][h]["n"]
                    if n == 0:
                        continue
                    ioff = gi["halves"][h]["ioff"]
                    it = idx_t[:, ioff:ioff + n // 16]
                    src = kv_il[0:HALF, :] if h == 0 else kv_il[HALF:POOL, :]
                    sb = 0 if h == 0 else n_lo_slots
                    nc.gpsimd.dma_gather(
                        out_ap=kvt[:, sb:sb + n // 128, :], in_ap=src,
                        idxs_ap=it, num_idxs=n, num_idxs_reg=n, elem_size=256,
                        transpose=False, single_packet=False)

                # --- K^T: PE transpose batches + PSUM->SBUF copies ---------
                ktT = ktp.tile([128, nslots * 128], dt.bfloat16, tag="ktT")
                for bi, s0 in enumerate(range(0, nslots, TB)):
                    nb = min(TB, nslots - s0)
                    kt_ps = pskt.tile([128, TB * 128], dt.bfloat16, tag="ktps")
                    for i in range(nb):
                        nc.tensor.transpose(kt_ps[:, 128 * i:128 * (i + 1)],
                                            kvt[:, s0 + i, 0:128], ident_t[:])
                    dst = ktT[:, 128 * s0:128 * (s0 + nb)]
                    if bi % CPMOD == CPACT:
                        nc.scalar.activation(dst, kt_ps[:, 0:128 * nb],
                                             mybir.ActivationFunctionType.Copy)
                    else:
                        nc.vector.tensor_copy(out=dst, in_=kt_ps[:, 0:128 * nb])

                # --- QK: scores^T per subslot into one PSUM bank ----------
                sc = pssc.tile([128, ncols], dt.float32, tag="sc")
                for si, (s, j, bid) in enumerate(gi["subs"]):
                    b = GOFF[g] + j
                    kT = ktT[:, 128 * s:128 * (s + 1)]
                    out = sc[:, 4 * si:4 * si + 4]
                    if bid >= 0:  # seed foreign partitions with -50 (K=1 mm)
                        nc.tensor.matmul(out,
                                         bias_t[0:1, 128 * bid:128 * bid + 128],
                                         ones4_t[0:1, :],
                                         start=True, stop=False)
                    nc.tensor.matmul(out, kT, qhi_t[:, 4 * b:4 * b + 4],
                                     start=(bid < 0), stop=False)
                    nc.tensor.matmul(out, kT, qlo_t[:, 4 * b:4 * b + 4],
                                     start=False, stop=True)

                # --- softmax numerator, straight to bf16 (scores are O(1))
                pt = ptp.tile([128, ncols], dt.bfloat16, tag="pt")
                pvs = pspv.tile([128, OC + ncols], dt.float32, tag="pvs")
                nc.scalar.activation(pt[:], sc[:],
                                     mybir.ActivationFunctionType.Exp)

                # --- PV (o^T accum, V-natural stationary) + sums ----------
                for j in range(GSIZES[g]):
                    rsubs = gi["req_subs"][j]
                    oc = G * j
                    if not rsubs:
                        nc.vector.memset(pvs[:, oc:oc + G], 0.0)
                        continue
                    last = len(rsubs) - 1
                    for kk, si in enumerate(rsubs):
                        s = gi["subs"][si][0]
                        nc.tensor.matmul(pvs[:, oc:oc + G], kvt[:, s, 128:256],
                                         pt[:, 4 * si:4 * si + 4],
                                         start=(kk == 0), stop=(kk == last))
                    for (s0, cnt) in gi["req_ranges"][j]:
                        nc.tensor.matmul(
                            pvs[0:1, OC + 4 * s0:OC + 4 * (s0 + cnt)],
                            ones_t[:, 0:1], pt[:, 4 * s0:4 * (s0 + cnt)],
                            start=True, stop=True)

                ostg = stgp.tile([128, OC], dt.float32, tag="ostg")
                nc.scalar.activation(ostg[:], pvs[:, 0:OC],
                                     mybir.ActivationFunctionType.Copy)
                so = gi["sbase"] - 4 * B
                nc.scalar.activation(sums_t[0:1, so:so + ncols],
                                     pvs[0:1, OC:OC + ncols],
                                     mybir.ActivationFunctionType.Copy)
                nc.sync.dma_start(out=o_dram[:, ob:ob + OC], in_=ostg[:])

            # all per-group sums in one trailing DMA (partition-0 row)
            nsum = out_w - 4 * B
            if nsum > 0:
                nc.sync.dma_start(out=o_dram[0:1, 4 * B:out_w],
                                  in_=sums_t[0:1, 0:nsum])

    nc.compile()
    return nc, info, bias_cols


def prepare(inputs):
    q = np.asarray(inputs["q"], np.float32)
    k = np.asarray(inputs["k"], np.float32)
    v = np.asarray(inputs["v"], np.float32)
    k_buffer = np.asarray(inputs["k_buffer"], np.float32)
    v_buffer = np.asarray(inputs["v_buffer"], np.float32)
    req_to_token = np.asarray(inputs["req_to_token"])
    req_pool_indices = np.asarray(inputs["req_pool_indices"])
    seq_lens = np.asarray(inputs["seq_lens"]).astype(np.int64)
    out_cache_loc = np.asarray(inputs["out_cache_loc"]).astype(np.int64)

    # store_kv_cache scatter (tiny: 32 rows) + per-request token lists
    kb = k_buffer.copy()
    vb = v_buffer.copy()
    kb[out_cache_loc] = k.reshape(B, HKV, D)
    vb[out_cache_loc] = v.reshape(B, HKV, D)
    tok = req_to_token[req_pool_indices]

    # spare (unreferenced) pool row per half -> zeroed pad target
    referenced = np.zeros(POOL, bool)
    for b in range(B):
        referenced[tok[b, :seq_lens[b]]] = True
    free_lo = np.flatnonzero(~referenced[:HALF])
    free_hi = np.flatnonzero(~referenced[HALF:])
    zero_row = [int(free_lo[0]) if len(free_lo) else -1,
                int(free_hi[0]) + HALF if len(free_hi) else -1]
    mask_halves = (zero_row[0] < 0, zero_row[1] < 0)

    # group 0: smallest requests (fast fill); trailing small groups get the
    # next-smallest (shortest drain last); the rest biggest-first in between
    asc = list(np.argsort(seq_lens, kind="stable"))
    s0 = GSIZES[0]
    ntail = 0
    while (ntail < GROUPS - 1 and GSIZES[GROUPS - 1 - ntail] <= 3):
        ntail += 1
    tsz = int(np.sum(GSIZES[GROUPS - ntail:])) if ntail else 0
    tail_pool = asc[s0:s0 + tsz][::-1]  # descending
    tail = []
    p = 0
    for g in range(GROUPS - ntail, GROUPS):
        tail.extend(sorted(tail_pool[p:p + GSIZES[g]]))
        p += GSIZES[g]
    mid = asc[s0 + tsz:][::-1]
    order = np.array(asc[:s0] + mid + tail, dtype=np.int64)

    meta = []
    idx_blocks = []
    for g in range(GROUPS):
        lo_secs, hi_secs = [], []
        for h in (0, 1):
            parts = []
            secs = lo_secs if h == 0 else hi_secs
            pad_idx = zero_row[h] - (0 if h == 0 else HALF)
            if pad_idx < 0:
                pad_idx = 0  # bias fallback half: any valid row
            for j in range(GSIZES[g]):
                b = int(order[GOFF[g] + j])
                t = tok[b, :seq_lens[b]].astype(np.int64)
                tl = t[t < HALF] if h == 0 else t[t >= HALF] - HALF
                secs.append(len(tl))
                parts.append(tl)
            full = np.concatenate(parts) if parts else np.zeros(0, np.int64)
            P = _pad128(len(full))
            if P:
                arr = np.full(P, pad_idx, np.int64)
                arr[:len(full)] = full
                # [16, n/16] wrap, replicated into all 8 GPSIMD-core stripes
                idx_blocks.append(
                    np.tile(arr.astype(np.int16).reshape(-1, 16).T, (8, 1)))
        meta.append((tuple(lo_secs), tuple(hi_secs)))
    meta = tuple(meta)
    if idx_blocks:
        idx_all = np.ascontiguousarray(np.concatenate(idx_blocks, axis=1))
    else:
        idx_all = np.zeros((128, 1), np.int16)

    key = (meta, mask_halves)
    if key not in _prog_cache:
        _prog_cache[key] = _build_program(meta, mask_halves)
    nc, info, bias_cols = _prog_cache[key]

    biasc = np.zeros((1, 128 * max(1, len(bias_cols))), BF16)
    for bi, (plo, phi) in enumerate(bias_cols):
        col = np.full(128, NEG, BF16)
        col[plo:phi] = 0.0
        biasc[0, 128 * bi:128 * bi + 128] = col

    ident = np.eye(128, dtype=BF16)
    in_maps = []
    for c in range(NCORES):
        k_hi = kb[:, c, :].astype(BF16)
        v_hi = vb[:, c, :].astype(BF16)
        qc = (q.reshape(B, HKV, G, D)[order, c] * SCALE).reshape(B * G, D)
        qT = np.ascontiguousarray(qc.T)
        q_hi = qT.astype(BF16)
        q_lo = (qT - q_hi.astype(np.float32)).astype(BF16)
        kv_core = np.concatenate([k_hi, v_hi], axis=1)
        for zr in zero_row:
            if zr >= 0:
                kv_core[zr] = 0
        im = {
            "kv_il": np.ascontiguousarray(kv_core),
            "qhiT": np.ascontiguousarray(q_hi),
            "qloT": np.ascontiguousarray(q_lo),
            "identd": ident,
            "biasc": biasc,
            "idx_all": idx_all,
        }
        in_maps.append(im)
    return nc, info, in_maps, order, mask_halves


def postprocess(results, info, order, mask_halves, cores=None):
    out = np.zeros((B, HQ, D), np.float32)
    for c in (cores if cores is not None else range(NCORES)):
        o_un = results[c]["o_un"]  # [128, W]: o^T cols + sums row segment
        for g in range(GROUPS):
            gi = info[g]
            ob = gi["obase"]
            sb = gi["sbase"]  # sums cols (partition-0 row)
            for j in range(GSIZES[g]):
                b = int(order[GOFF[g] + j])
                stot = np.zeros(G, np.float64)
                for (s0, cnt) in gi["req_ranges"][j]:
                    seg = o_un[0, sb + 4 * s0:sb + 4 * (s0 + cnt)]
                    stot += seg.astype(np.float64).reshape(cnt, G).sum(axis=0)
                stot -= gi["req_pads"][j]  # zero-row pads: exp(0)=1 each
                ov = o_un[:, ob + G * j:ob + G * (j + 1)]  # [128 d, G]
                with np.errstate(divide="ignore", invalid="ignore"):
                    out[b, c * G:(c + 1) * G, :] = (ov / stot[None, :]).T
    return out.reshape(B, HQ * D).astype(np.float32)


def kernel(**inputs):
    global LAST_RESULT
    nc, info, in_maps, order, mask_halves = prepare(inputs)
    res = run_bass_kernel_spmd(nc, in_maps, core_ids=list(range(NCORES)),
                               trace=False)
    LAST_RESULT = res
    return postprocess(res.results, info, order, mask_halves)
